# revision 4
# baseline (speedup 1.0000x reference)
"""Trainium2 Bass kernel for zonotope AbstractRelu (nn_AbstractRelu_76751065579631).

Problem: x [E=512, D1=4096, D2=16] f32. Per neuron column n (N = D1*D2 = 65536):
    sum_abs[n] = sum_{e>=1} |x[e, n]|
    lb = x[0] - sum_abs ; ub = x[0] + sum_abs
    scale = (ub > 0) * (1 - min(lb, 0))
    out[0]   = scale * (x[0] - min(lb, 0)/2)
    out[1:]  = scale * x[1:]
(algebraically identical to the reference's crossing/dead formulation)

Sharding: neuron columns split contiguously across 8 cores (8192 each), no
communication.

Precision/traffic: tolerance is 2e-2 rel err; the 511 error rows are carried
in bf16 end-to-end (host casts input, device stores bf16, host casts back),
while the center row travels f32 (it holds ~98% of output energy and decides
the crossing/dead classification). This halves HBM traffic per core vs f32:
~16.9 MB, a ~47 us floor at the ~358 GB/s/core DMA roofline.

Layout: error terms sit on partitions (4 blocks of 128; the host zeroes row 0
of the bf16 tensor so no reduce mask is needed). |x| is taken per block in
bf16 -- split 2 blocks on ACT / 2 on DVE so neither engine gates the DMA
stream -- and the cross-partition reduction runs as PSUM-accumulated
ones-matmuls on TensorE (bf16 = full PE rate).

The per-neuron scale path runs per SUPER-chunk of SC=4 chunks (4096 cols) on
a [128, 32] repartitioned layout: the [1,4096]<->[128,32] repartition DMAs
move 128B runs per partition instead of the 32B runs a per-chunk scale path
would use (measured: those tiny packets dominated DMA-queue overhead).
Scale is broadcast back across partitions with a K=1 ones matmul into PSUM.

Pipeline: super-chunk j's loads/abs/reduce are emitted, then super-chunk
j-1's broadcast/multiply/stores, then j's scale math. The in-order DVE
stream (abs(j) before mult(j-1)) naturally sequences all loads ahead of all
stores, so DMA streams continuously: loads 0..23us, stores trail to ~50us.
"""

import os

import numpy as np

E = 512
D1 = 4096
D2 = 16
N = D1 * D2          # 65536 neurons
NCORES = 8
COLS = N // NCORES   # 8192 neuron columns per core
W = 1024             # chunk width
SC = 4               # chunks per super-chunk (scale-path granularity)

LAST_EXEC_TIME_NS = None

_CACHE = {}


def _emit(tc, oe_ap, oc_ap, xe_ap, xc_ap, W, SC):
    import concourse.mybir as mybir

    nc = tc.nc
    f32 = mybir.dt.float32
    bf16 = mybir.dt.bfloat16
    Alu = mybir.AluOpType
    Act = mybir.ActivationFunctionType

    e_total, cols = xe_ap.shape
    NB = e_total // 128          # e-blocks of 128 partitions
    NCH = cols // W              # chunks
    NSC = NCH // SC              # super-chunks
    SCW = SC * W                 # super-chunk width
    WP = SCW // 128              # repartitioned free width

    # DRAM views [NB, 128, cols]
    x_blk = xe_ap.rearrange("(b p) n -> b p n", p=128)
    o_blk = oe_ap.rearrange("(b p) n -> b p n", p=128)

    with (
        tc.tile_pool(name="const", bufs=1) as const_pool,
        tc.tile_pool(name="x", bufs=10) as x_pool,
        tc.tile_pool(name="abs", bufs=8) as abs_pool,
        tc.tile_pool(name="row", bufs=2) as row_pool,
        tc.tile_pool(name="small", bufs=2) as small_pool,
        tc.tile_pool(name="psum_s", bufs=2, space="PSUM") as psum_s_pool,
        tc.tile_pool(name="psum_b", bufs=2, space="PSUM") as psum_b_pool,
    ):
        ones_row = const_pool.tile([1, 128], bf16, tag="ones_row")
        nc.vector.memset(ones_row[:], 1.0)
        ones_col = const_pool.tile([128, 1], bf16, tag="ones_col")
        nc.vector.memset(ones_col[:], 1.0)

        def pieces(Wk):
            """512-wide PSUM bank pieces covering a Wk-wide chunk."""
            return [(ps, min(512, Wk - ps)) for ps in range(0, Wk, 512)]

        def front(cs, s_sc, koff):
            """Loads (SP+ACT rings), |x| in bf16 (2 blocks ACT, 2 DVE),
            partition-sum matmuls (PE), psum -> s_sc row copy (ACT)."""
            st = {"cs": cs}
            xt = x_pool.tile([128, NB * W], bf16, tag="x")
            blk = [xt[:, W * b:W * (b + 1)] for b in range(NB)]
            for b in range(NB):
                # split loads across the SP and ACT HWDGE rings so loads get
                # a bigger share of the DMA engines' ring arbitration
                eng = nc.sync if b < 2 else nc.scalar
                eng.dma_start(out=blk[b], in_=x_blk[b, :, cs:cs + W])

            # |x| per block in bf16; ACT and DVE split the work.
            psum_s = psum_s_pool.tile([1, W], f32, tag="s")
            ats = []
            for b in range(NB):
                at = abs_pool.tile([128, W], bf16, tag="a")
                if b < 2:
                    nc.scalar.activation(at[:], blk[b], Act.Abs)
                else:
                    # DVE abs: max(-x, x) in one scalar_tensor_tensor
                    nc.vector.scalar_tensor_tensor(
                        at[:], in0=blk[b], scalar=-1.0, in1=blk[b],
                        op0=Alu.mult, op1=Alu.max,
                    )
                ats.append(at)
            for ps, pw in pieces(W):
                for b in range(NB):
                    nc.tensor.matmul(
                        psum_s[0:1, ps:ps + pw],
                        lhsT=ones_col[:],
                        rhs=ats[b][:, ps:ps + pw],
                        start=(b == 0),
                        stop=(b == NB - 1),
                    )
            # free psum_s early: copy into the super-chunk row (ACT)
            nc.scalar.copy(s_sc[0:1, koff * W:(koff + 1) * W], psum_s[:])
            st.update(xt=xt, blk=blk)
            return st

        def scale_phase(j, s_sc):
            """Per-super-chunk scale math on the [128, WP] repartition."""
            cs = j * SCW
            # center row: DRAM row -> [128, WP] (SP ring, no deps)
            c_t = small_pool.tile([128, WP], f32, tag="ct")
            nc.sync.dma_start(out=c_t[:], in_=xc_ap[0:1, cs:cs + SCW])
            # sum_abs row -> [128, WP] (ACT ring, right after the copies it
            # depends on)
            s_t = small_pool.tile([128, WP], f32, tag="st")
            nc.scalar.dma_start(out=s_t[:], in_=s_sc[:])

            # per-neuron math on [128, WP] (DVE, all tiny)
            lb = small_pool.tile([128, WP], f32, tag="lb")
            nc.vector.tensor_sub(lb[:], c_t[:], s_t[:])
            ub = small_pool.tile([128, WP], f32, tag="ub")
            nc.vector.tensor_add(ub[:], c_t[:], s_t[:])
            min0 = small_pool.tile([128, WP], f32, tag="min0")
            nc.vector.tensor_scalar_min(min0[:], lb[:], 0.0)
            alpha = small_pool.tile([128, WP], f32, tag="alpha")
            nc.vector.tensor_scalar(alpha[:], min0[:], -1.0, 1.0, Alu.mult,
                                    Alu.add)
            gt = small_pool.tile([128, WP], f32, tag="gt")
            nc.vector.tensor_scalar(gt[:], ub[:], 0.0, None, Alu.is_gt)
            scale = small_pool.tile([128, WP], f32, tag="scale")
            nc.vector.tensor_mul(scale[:], alpha[:], gt[:])
            # bf16 copy of scale for the broadcast matmul
            scale_bf = small_pool.tile([128, WP], bf16, tag="scalebf")
            nc.vector.tensor_mul(scale_bf[:], alpha[:], gt[:])

            # scale back to row layout for the K=1 broadcast matmuls
            # (GpSimd/SWDGE queue: keeps the ACT sequencer free)
            scale_row = row_pool.tile([1, SCW], bf16, tag="scrow")
            nc.gpsimd.dma_start(out=scale_row[:], in_=scale_bf[:])

            t1 = small_pool.tile([128, WP], f32, tag="t1")
            nc.vector.scalar_tensor_tensor(
                t1[:], in0=min0[:], scalar=-0.5, in1=c_t[:],
                op0=Alu.mult, op1=Alu.add,
            )
            cnew = small_pool.tile([128, WP], f32, tag="cnew")
            nc.vector.tensor_mul(cnew[:], t1[:], scale[:])
            # center output: [128, WP] -> DRAM row (reverse repartition)
            nc.gpsimd.dma_start(out=oc_ap[0:1, cs:cs + SCW], in_=cnew[:])
            return scale_row

        def backmul(st, scale_row, koff):
            """Broadcast scale across partitions (K=1 ones matmul) and
            multiply the 4 x-blocks in place (DVE, bf16 out)."""
            blk = st["blk"]
            psum_b = psum_b_pool.tile([128, W], f32, tag="b")
            for ps, pw in pieces(W):
                nc.tensor.matmul(
                    psum_b[:, ps:ps + pw],
                    lhsT=ones_row[:],
                    rhs=scale_row[0:1, koff * W + ps:koff * W + ps + pw],
                    start=True,
                    stop=True,
                )
            for b in range(NB):
                nc.vector.tensor_mul(blk[b], blk[b], psum_b[:])

        def back2(st):
            """Per-block stores (Pool SWDGE queue)."""
            cs, blk = st["cs"], st["blk"]
            for b in range(NB):
                nc.gpsimd.dma_start(out=o_blk[b, :, cs:cs + W], in_=blk[b])

        prev = None  # (stages, scale_row) of super-chunk j-1
        for j in range(NSC):
            s_sc = row_pool.tile([1, SCW], f32, tag="s_sc")
            stages = [front((j * SC + k) * W, s_sc, k) for k in range(SC)]
            if prev is not None:
                pstages, pscale = prev
                for k in range(SC):
                    backmul(pstages[k], pscale, k)
                    back2(pstages[k])
            scale_row = scale_phase(j, s_sc)
            prev = (stages, scale_row)
        pstages, pscale = prev
        for k in range(SC):
            backmul(pstages[k], pscale, k)
            back2(pstages[k])


def build(cols=COLS, e_total=E, w=W, sc=SC):
    """Build + compile the per-core Bass program (cached)."""
    key = (cols, e_total, w, sc)
    if key in _CACHE:
        return _CACHE[key]

    from concourse import bacc
    import concourse.mybir as mybir
    from concourse.tile import TileContext

    nc = bacc.Bacc("TRN2", target_bir_lowering=False, debug=False,
                   num_devices=NCORES)
    xe_ap = nc.dram_tensor("xe", [e_total, cols], mybir.dt.bfloat16,
                           kind="ExternalInput").ap()
    xc_ap = nc.dram_tensor("xc", [1, cols], mybir.dt.float32,
                           kind="ExternalInput").ap()
    oe_ap = nc.dram_tensor("oe", [e_total, cols], mybir.dt.bfloat16,
                           kind="ExternalOutput").ap()
    oc_ap = nc.dram_tensor("oc", [1, cols], mybir.dt.float32,
                           kind="ExternalOutput").ap()
    with TileContext(nc) as tc:
        _emit(tc, oe_ap, oc_ap, xe_ap, xc_ap, w, sc)
    nc.compile()
    _CACHE[key] = nc
    return nc


def _ensure_ntff_hook():
    """Install the axon NTFF profile hook when the image's antenv lacks it."""
    import sys
    import types

    try:
        from antenv.axon_hooks import get_axon_ntff_profile_hook  # noqa: F401
        return
    except ImportError:
        pass

    mod = types.ModuleType("antenv.axon_hooks")
    mod._hook = None

    def set_axon_ntff_profile_hook(h):
        mod._hook = h

    def get_axon_ntff_profile_hook():
        return mod._hook

    mod.set_axon_ntff_profile_hook = set_axon_ntff_profile_hook
    mod.get_axon_ntff_profile_hook = get_axon_ntff_profile_hook
    sys.modules["antenv.axon_hooks"] = mod
    import antenv

    antenv.axon_hooks = mod
    try:
        from trn_agent_boot.trn_boot import _ntff_profile_via_ctypes

        set_axon_ntff_profile_hook(
            _ntff_profile_via_ctypes("/opt/axon/libaxon_pjrt.so")
        )
    except Exception:
        pass


def kernel(x):
    global LAST_EXEC_TIME_NS
    import ml_dtypes
    from concourse import bass_utils

    nc = build()
    xf = np.asarray(x, dtype=np.float32).reshape(E, N)
    xe = xf.astype(ml_dtypes.bfloat16)
    xe[0] = 0  # center row excluded from the |.| reduce
    in_maps = []
    for c in range(NCORES):
        sl = slice(c * COLS, (c + 1) * COLS)
        in_maps.append({
            "xe": np.ascontiguousarray(xe[:, sl]),
            "xc": np.ascontiguousarray(xf[0:1, sl]),
        })
    trace = bool(int(os.environ.get("KERNEL_TRACE", "0")))
    if trace:
        _ensure_ntff_hook()
        # Sandboxed container: keep profile artifacts local.
        bass_utils.upload_artifacts = lambda tmpdir: tmpdir
    res = bass_utils.run_bass_kernel_spmd(
        nc, in_maps, core_ids=list(range(NCORES)), trace=trace
    )
    LAST_EXEC_TIME_NS = res.exec_time_ns
    out = np.empty((E, N), dtype=np.float32)
    for c in range(NCORES):
        sl = slice(c * COLS, (c + 1) * COLS)
        out[1:, sl] = res.results[c]["oe"][1:].astype(np.float32)
        out[0, sl] = res.results[c]["oc"][0]
    return out.reshape(E, D1, D2)


# revision 11
# speedup vs baseline: 1.3113x; 1.3113x over previous
"""Trainium2 Bass kernel for zonotope AbstractRelu (nn_AbstractRelu_76751065579631).

Problem: x [E=512, D1=4096, D2=16] f32. Per neuron column n (N = D1*D2 = 65536):
    sum_abs[n] = sum_{e>=1} |x[e, n]|
    lb = x[0] - sum_abs ; ub = x[0] + sum_abs
    scale = (ub > 0) * (1 - min(lb, 0))
    out[0]   = scale * (x[0] - min(lb, 0)/2)
    out[1:]  = scale * x[1:]
(algebraically identical to the reference's crossing/dead formulation)

Sharding: neuron columns split contiguously across 8 cores (8192 each), no
communication.

Precision/traffic: tolerance is 2e-2 rel err; the 511 error rows are carried
in bf16 end-to-end (host casts input, device stores bf16, host casts back),
while the center row travels f32 (it holds ~98% of output energy and decides
the crossing/dead classification). This halves HBM traffic per core vs f32:
~16.9 MB, a ~47 us floor at the ~358 GB/s/core DMA roofline.

Layout: error terms sit on partitions (4 blocks of 128; the host zeroes row 0
of the bf16 tensor so no reduce mask is needed). |x| is taken per block in
bf16 -- split 2 blocks on ACT / 2 on DVE so neither engine gates the DMA
stream -- and the cross-partition reduction runs as PSUM-accumulated
ones-matmuls on TensorE (bf16 = full PE rate).

The per-neuron scale path runs per SUPER-chunk of SC=4 chunks (4096 cols) on
a [128, 32] repartitioned layout: the [1,4096]<->[128,32] repartition DMAs
move 128B runs per partition instead of the 32B runs a per-chunk scale path
would use (measured: those tiny packets dominated DMA-queue overhead).
Scale is broadcast back across partitions with a K=1 ones matmul into PSUM.

Pipeline: super-chunk j's loads/abs/reduce are emitted, then super-chunk
j-1's broadcast/multiply/stores, then j's scale math. The in-order DVE
stream (abs(j) before mult(j-1)) naturally sequences all loads ahead of all
stores, so DMA streams continuously: loads 0..23us, stores trail to ~50us.
"""

import os

import numpy as np

E = 512
D1 = 4096
D2 = 16
N = D1 * D2          # 65536 neurons
NCORES = 8
COLS = N // NCORES   # 8192 neuron columns per core
W = 1024             # chunk width
SC = 4               # chunks per super-chunk (scale-path granularity)

LAST_EXEC_TIME_NS = None

_CACHE = {}


def _emit(tc, oe_ap, oc_ap, xe_ap, xc_ap, W, SC):
    import concourse.mybir as mybir

    nc = tc.nc
    f32 = mybir.dt.float32
    bf16 = mybir.dt.bfloat16
    Alu = mybir.AluOpType
    Act = mybir.ActivationFunctionType

    e_total, cols = xe_ap.shape
    NB = e_total // 128          # e-blocks of 128 partitions
    NCH = cols // W              # chunks
    NSC = NCH // SC              # super-chunks
    SCW = SC * W                 # super-chunk width
    WP = SCW // 128              # repartitioned free width

    # DRAM views: partition-major [128, NB, cols] so ONE dma_start moves a
    # whole chunk (all NB e-blocks) -- each dma_start costs the issuing
    # engine ~650ns, so consolidation matters.
    x_pbn = xe_ap.rearrange("(b p) n -> p b n", p=128)
    o_pbn = oe_ap.rearrange("(b p) n -> p b n", p=128)

    # abs split point inside the [128, NB*W] chunk tile: first ABS_ACT
    # columns on ACT, rest on DVE (balances the two engines' budgets,
    # with ACT also carrying the psum_b -> bf16 copies)
    ABS_ACT = (NB * W * 9) // 16

    with (
        tc.tile_pool(name="const", bufs=1) as const_pool,
        tc.tile_pool(name="x", bufs=8) as x_pool,
        tc.tile_pool(name="abs", bufs=4) as abs_pool,
        tc.tile_pool(name="row", bufs=2) as row_pool,
        tc.tile_pool(name="small", bufs=2) as small_pool,
        tc.tile_pool(name="bc", bufs=3) as bc_pool,
        tc.tile_pool(name="psum_s", bufs=2, space="PSUM") as psum_s_pool,
        tc.tile_pool(name="psum_b", bufs=2, space="PSUM") as psum_b_pool,
    ):
        ones_row = const_pool.tile([1, 128], bf16, tag="ones_row")
        nc.vector.memset(ones_row[:], 1.0)
        ones_col = const_pool.tile([128, 1], bf16, tag="ones_col")
        nc.vector.memset(ones_col[:], 1.0)

        def pieces(Wk):
            """512-wide PSUM bank pieces covering a Wk-wide chunk."""
            return [(ps, min(512, Wk - ps)) for ps in range(0, Wk, 512)]

        def front(cs, s_sc, koff):
            """One chunk load (SP ring), |x| in bf16 (split ACT/DVE),
            partition-sum matmuls (PE), psum -> s_sc row copy (ACT)."""
            st = {"cs": cs}
            xt = x_pool.tile([128, NB * W], bf16, tag="x")
            nc.sync.dma_start(out=xt[:], in_=x_pbn[:, :, cs:cs + W])

            # |x| in bf16, one sliced instruction per engine
            at = abs_pool.tile([128, NB * W], bf16, tag="a")
            nc.scalar.activation(at[:, 0:ABS_ACT], xt[:, 0:ABS_ACT], Act.Abs)
            # DVE abs: max(-x, x) in one scalar_tensor_tensor
            nc.vector.scalar_tensor_tensor(
                at[:, ABS_ACT:], in0=xt[:, ABS_ACT:], scalar=-1.0,
                in1=xt[:, ABS_ACT:], op0=Alu.mult, op1=Alu.max,
            )
            psum_s = psum_s_pool.tile([1, W], f32, tag="s")
            for ps, pw in pieces(W):
                for b in range(NB):
                    nc.tensor.matmul(
                        psum_s[0:1, ps:ps + pw],
                        lhsT=ones_col[:],
                        rhs=at[:, b * W + ps:b * W + ps + pw],
                        start=(b == 0),
                        stop=(b == NB - 1),
                    )
            # free psum_s early: copy into the super-chunk row (ACT)
            nc.scalar.copy(s_sc[0:1, koff * W:(koff + 1) * W], psum_s[:])
            st.update(xt=xt)
            return st

        def scale_phase(j, s_sc):
            """Per-super-chunk scale math on the [128, WP] repartition."""
            cs = j * SCW
            # center row: DRAM row -> [128, WP] (ACT ring, no deps)
            c_t = small_pool.tile([128, WP], f32, tag="ct")
            nc.scalar.dma_start(out=c_t[:], in_=xc_ap[0:1, cs:cs + SCW])
            # sum_abs row -> [128, WP] (ACT ring, right after the copies it
            # depends on)
            s_t = small_pool.tile([128, WP], f32, tag="st")
            nc.scalar.dma_start(out=s_t[:], in_=s_sc[:])

            # per-neuron math on [128, WP] (DVE, all tiny)
            lb = small_pool.tile([128, WP], f32, tag="lb")
            nc.vector.tensor_sub(lb[:], c_t[:], s_t[:])
            ub = small_pool.tile([128, WP], f32, tag="ub")
            nc.vector.tensor_add(ub[:], c_t[:], s_t[:])
            min0 = small_pool.tile([128, WP], f32, tag="min0")
            nc.vector.tensor_scalar_min(min0[:], lb[:], 0.0)
            alpha = small_pool.tile([128, WP], f32, tag="alpha")
            nc.vector.tensor_scalar(alpha[:], min0[:], -1.0, 1.0, Alu.mult,
                                    Alu.add)
            gt = small_pool.tile([128, WP], f32, tag="gt")
            nc.vector.tensor_scalar(gt[:], ub[:], 0.0, None, Alu.is_gt)
            scale = small_pool.tile([128, WP], f32, tag="scale")
            nc.vector.tensor_mul(scale[:], alpha[:], gt[:])
            # bf16 copy of scale for the broadcast matmul
            scale_bf = small_pool.tile([128, WP], bf16, tag="scalebf")
            nc.vector.tensor_mul(scale_bf[:], alpha[:], gt[:])

            # scale back to row layout for the K=1 broadcast matmuls
            # (GpSimd/SWDGE queue: keeps the ACT sequencer free)
            scale_row = row_pool.tile([1, SCW], bf16, tag="scrow")
            nc.gpsimd.dma_start(out=scale_row[:], in_=scale_bf[:])

            t1 = small_pool.tile([128, WP], f32, tag="t1")
            nc.vector.scalar_tensor_tensor(
                t1[:], in0=min0[:], scalar=-0.5, in1=c_t[:],
                op0=Alu.mult, op1=Alu.add,
            )
            cnew = small_pool.tile([128, WP], f32, tag="cnew")
            nc.vector.tensor_mul(cnew[:], t1[:], scale[:])
            # center output: [128, WP] -> DRAM row (reverse repartition)
            nc.gpsimd.dma_start(out=oc_ap[0:1, cs:cs + SCW], in_=cnew[:])
            return scale_row

        def backmul(st, scale_row, koff):
            """Broadcast scale across partitions (K=1 ones matmul), copy to
            bf16 (Pool), multiply the 4 x-blocks in place (DVE, pure bf16 =
            2x rate)."""
            xt = st["xt"]
            psum_b = psum_b_pool.tile([128, W], f32, tag="b")
            for ps, pw in pieces(W):
                nc.tensor.matmul(
                    psum_b[:, ps:ps + pw],
                    lhsT=ones_row[:],
                    rhs=scale_row[0:1, koff * W + ps:koff * W + ps + pw],
                    start=True,
                    stop=True,
                )
            bc = bc_pool.tile([128, W], bf16, tag="bc")
            nc.scalar.copy(bc[:], psum_b[:])
            for b in range(NB):
                nc.vector.tensor_mul(xt[:, b * W:(b + 1) * W],
                                     xt[:, b * W:(b + 1) * W], bc[:])

        def back2(st):
            """One consolidated chunk store (Pool SWDGE queue)."""
            cs, xt = st["cs"], st["xt"]
            nc.gpsimd.dma_start(out=o_pbn[:, :, cs:cs + W], in_=xt[:])

        # Fine-grained 3-deep chunk pipeline: front(k) | scale at each
        # super-chunk boundary | backmul/back2(k-SC). DVE interleaves
        # next-super-chunk abs with current-super-chunk multiplies, so the
        # store stream starts while loads are still going.
        stages = []
        scale_rows = {}
        s_sc = None
        for k in range(NCH):
            j = k // SC
            if k % SC == 0:
                s_sc = row_pool.tile([1, SCW], f32, tag="s_sc")
            stages.append(front(k * W, s_sc, k % SC))
            if k % SC == SC - 1:
                scale_rows[j] = scale_phase(j, s_sc)
            if k >= SC:
                backmul(stages[k - SC], scale_rows[(k - SC) // SC],
                        (k - SC) % SC)
                back2(stages[k - SC])
        for k in range(NCH - SC, NCH):
            backmul(stages[k], scale_rows[k // SC], k % SC)
            back2(stages[k])


def build(cols=COLS, e_total=E, w=W, sc=SC):
    """Build + compile the per-core Bass program (cached)."""
    key = (cols, e_total, w, sc)
    if key in _CACHE:
        return _CACHE[key]

    from concourse import bacc
    import concourse.mybir as mybir
    from concourse.tile import TileContext

    nc = bacc.Bacc("TRN2", target_bir_lowering=False, debug=False,
                   num_devices=NCORES)
    xe_ap = nc.dram_tensor("xe", [e_total, cols], mybir.dt.bfloat16,
                           kind="ExternalInput").ap()
    xc_ap = nc.dram_tensor("xc", [1, cols], mybir.dt.float32,
                           kind="ExternalInput").ap()
    oe_ap = nc.dram_tensor("oe", [e_total, cols], mybir.dt.bfloat16,
                           kind="ExternalOutput").ap()
    oc_ap = nc.dram_tensor("oc", [1, cols], mybir.dt.float32,
                           kind="ExternalOutput").ap()
    with TileContext(nc) as tc:
        _emit(tc, oe_ap, oc_ap, xe_ap, xc_ap, w, sc)
    nc.compile()
    _CACHE[key] = nc
    return nc


def _ensure_ntff_hook():
    """Install the axon NTFF profile hook when the image's antenv lacks it."""
    import sys
    import types

    try:
        from antenv.axon_hooks import get_axon_ntff_profile_hook  # noqa: F401
        return
    except ImportError:
        pass

    mod = types.ModuleType("antenv.axon_hooks")
    mod._hook = None

    def set_axon_ntff_profile_hook(h):
        mod._hook = h

    def get_axon_ntff_profile_hook():
        return mod._hook

    mod.set_axon_ntff_profile_hook = set_axon_ntff_profile_hook
    mod.get_axon_ntff_profile_hook = get_axon_ntff_profile_hook
    sys.modules["antenv.axon_hooks"] = mod
    import antenv

    antenv.axon_hooks = mod
    try:
        from trn_agent_boot.trn_boot import _ntff_profile_via_ctypes

        set_axon_ntff_profile_hook(
            _ntff_profile_via_ctypes("/opt/axon/libaxon_pjrt.so")
        )
    except Exception:
        pass


def kernel(x):
    global LAST_EXEC_TIME_NS
    import ml_dtypes
    from concourse import bass_utils

    nc = build()
    xf = np.asarray(x, dtype=np.float32).reshape(E, N)
    xe = xf.astype(ml_dtypes.bfloat16)
    xe[0] = 0  # center row excluded from the |.| reduce
    in_maps = []
    for c in range(NCORES):
        sl = slice(c * COLS, (c + 1) * COLS)
        in_maps.append({
            "xe": np.ascontiguousarray(xe[:, sl]),
            "xc": np.ascontiguousarray(xf[0:1, sl]),
        })
    trace = bool(int(os.environ.get("KERNEL_TRACE", "0")))
    if trace:
        _ensure_ntff_hook()
        # Sandboxed container: keep profile artifacts local.
        bass_utils.upload_artifacts = lambda tmpdir: tmpdir
    res = bass_utils.run_bass_kernel_spmd(
        nc, in_maps, core_ids=list(range(NCORES)), trace=trace
    )
    LAST_EXEC_TIME_NS = res.exec_time_ns
    out = np.empty((E, N), dtype=np.float32)
    for c in range(NCORES):
        sl = slice(c * COLS, (c + 1) * COLS)
        out[1:, sl] = res.results[c]["oe"][1:].astype(np.float32)
        out[0, sl] = res.results[c]["oc"][0]
    return out.reshape(E, D1, D2)
